# revision 1
# baseline (speedup 1.0000x reference)
"""Trainium2 Bass kernel for LorentzBatchNorm (DMA-roofline version).

Math: for points x on the unit hyperboloid (linner(x,x) = -1) and the
normalized centroid `mean` (linner(mean,mean) = -1), the whole module
collapses per point to a rank-1 update:

  alpha = -linner(mean, x)            (one 128-dot per point)
  linner(u,u) = alpha^2 - 1           (u = x - alpha*mean; no 2nd reduction)
  d = arccosh(alpha) = ||x_T||        (Frechet var = mean of d)
  With beta = e0: transport to origin just zeroes channel 0, nu = g*d with
  g = gamma/(var+eps), and

  y[c] = A*x[c] - B*mean[c]  (c >= 1),   y[0] = cosh(nu)
  A = sinh(nu)/sqrt(alpha^2-1)
  B = A * (alpha + u0/(1+mean0)),  u0 = x0 - alpha*mean0

Performance model (TimelineSim): DMA_ENGINES is a single 360 GB/s device;
per-core traffic is 16.78 MB in + 16.78 MB out = 93.2 us of transfer. The
kernel keeps that device saturated: all 8 sample loads are issued up-front
into 8 static SBUF buffers, y is computed in-place over x, and stores
stream out in 8-tile chunks as soon as each chunk's tiles are done.
Compute is balanced so each engine stays under ~9 us/sample (< the
11.65 us/sample DMA cadence). Emission is stage-skewed with the scalar
chain (cross-engine ping-pong) sandwiched between bulk ready work so the
4-deep per-engine wait queues can bypass stalled instructions.
"""

import sys

if "/opt/trn_rl_repo" not in sys.path:
    sys.path.insert(0, "/opt/trn_rl_repo")

from contextlib import ExitStack

import numpy as np

import concourse.bass as bass
import concourse.tile as tile
from concourse import mybir
from concourse.vector_clock import ScopedClock

f32 = mybir.dt.float32
bf16 = mybir.dt.bfloat16
ALU = mybir.AluOpType
ACTF = mybir.ActivationFunctionType
X_AXIS = mybir.AxisListType.X

BS, H, W, C = 64, 64, 64, 128
N = H * W  # 4096 points per sample
NCORES = 8
SPB = BS // NCORES  # samples per core
NT = N // 128  # 32 tiles of 128 points
EPS = 1e-5
ACLIP = 1.0 + 1e-7
LN_HALF = float(np.log(0.5))

# --- engine assignment knobs ----------------------------------------------
# conv chunks (4 x 8 tiles): 'd'=DVE, 'a'=ACT, 'p'=Pool
CONV_ENGINES = "apaa"
# xbt transpose-group copies (4 x 8 tiles)
XBT_ENGINES = "aaaa"
# y pass, per tile (32). Each char picks the (amneg, finish) engine combo:
#  'A' = DVE tmp-STT + ACT scale-copy finish
#  'D' = DVE amneg (tensor_scalar, 2x) + DVE y-STT
#  'M' = DVE amneg + Pool y-STT
#  'N' = Pool amneg + DVE y-STT
#  'P' = Pool amneg + Pool y-STT
Y_MODES = "DDNNNNNN" "DDNNNNNN" "DDNNNNNN" "ADNNNNNN"
S0_SPLIT = 4  # load chunks for sample 0 (faster pipeline start)
S1_SPLIT = 1
STORE_SPLIT = 4  # tiles per store DMA (8 or 4)
# warmup patterns for sample 0 (engines are idle; spread for latency)
WARM_CONV = "dapd"
WARM_XBT = "dddd"


# ---------------------------------------------------------------------------
# Tile drain patch: the walrus CoreV3 codegen in this container accepts only
# one sync-wait per CTRL (Drain) instruction, but Tile's final drain piles the
# whole global clock onto a single Drain. Split across chained SP drains.
def _patched_drain_and_barrier(self, tick_clock, wait_clock):
    nc = self.nc
    drain_inst = nc.sync.drain()
    wait_clock.add_sem_waits(
        drain_inst.ins, ScopedClock({None: tick_clock.global_clock})
    )
    si = drain_inst.ins.sync_info
    waits = list(si.on_wait or [])
    if len(waits) > 1:
        si.on_wait = waits[:1]
        for w in waits[1:]:
            d2 = nc.sync.drain()
            si2 = d2.ins.sync_info
            if si2 is None:
                d2.ins.sync_info = mybir.SyncInfo(on_wait=[w], on_update=[])
            else:
                si2.on_wait = [w]
    nc.all_engine_barrier()
    assert self.sems is not None
    popped = nc._tile_sem_poison_stack.pop()
    assert popped is self._sem_poison
    nc.clear_and_free_semaphores(list(self.sems.allocated().values()))
    nc.all_engine_barrier()


_orig_lower_ordered_insts = tile.TileContext._lower_ordered_insts
_wsplit_counter = [0]


def _patched_lower_ordered_insts(self, ordered):
    """Walrus here allows only one sync-wait per instruction; hoist extra
    waits onto same-engine NoOps inserted just before the instruction."""
    maxw = 1
    for insts in ordered.values():
        out = []
        for inst in insts:
            si = inst.sync_info
            waits = list(si.on_wait) if si is not None and si.on_wait else []
            if len(waits) > maxw:
                extra, keep = waits[:-maxw], waits[-maxw:]
                for i in range(0, len(extra), maxw):
                    _wsplit_counter[0] += 1
                    nop = mybir.InstNoOp(
                        name=f"wsplit-{_wsplit_counter[0]}",
                        engine=inst.engine,
                        ins=[],
                        outs=[],
                        sync_info=mybir.SyncInfo(
                            on_wait=extra[i : i + maxw], on_update=[]
                        ),
                    )
                    out.append(nop)
                si.on_wait = keep
            out.append(inst)
        insts[:] = out
    return _orig_lower_ordered_insts(self, ordered)


def _install_tile_patch():
    tile.TileContext._drain_and_barrier = _patched_drain_and_barrier
    tile.TileContext._lower_ordered_insts = _patched_lower_ordered_insts


# ---------------------------------------------------------------------------


class _Ctx:
    """Shared build-time state (pools, constants, per-sample tiles)."""


def _emit_conv_chunk(nc, cx, s, i):
    """Sample s: bf16 convert of 8-tile chunk i + its 8 centroid matmuls."""
    x_sb = cx.x_sbs[s]
    eng = {"d": nc.vector, "a": nc.scalar, "p": nc.gpsimd}
    if i == 0:
        cx.xb[s] = cx.xbpool.tile([128, NT, C], bf16, tag="xb", name="xb")
        cx.psSs[s] = cx.psS.tile([128, C], f32, tag="psS", name="psS")
    xb = cx.xb[s]
    src = x_sb[:, 8 * i : 8 * (i + 1), :].rearrange("p a c -> p (a c)")
    dst = xb[:, 8 * i : 8 * (i + 1), :].rearrange("p a c -> p (a c)")
    e = (WARM_CONV if s <= 1 else CONV_ENGINES)[i]
    if e == "a":
        nc.scalar.copy(dst, src)
    else:
        eng[e].tensor_copy(dst, src)
    psS = cx.psSs[s]
    for k in range(8):
        t = 8 * i + k
        nc.tensor.matmul(
            psS, cx.ones, xb[:, t, :], start=(t == 0), stop=(t == NT - 1)
        )


def _emit_stats(nc, cx, s):
    """Sample s: x0 column, centroid stats -> Mrow, i1p, Wbcol."""
    x_sb = cx.x_sbs[s]

    # ---- x0 column (read before in-place y overwrites)
    x0 = cx.chain.tile([128, NT], f32, tag="x0")
    nc.gpsimd.tensor_copy(x0, x_sb[:, :, 0:1].rearrange("p t c -> p (t c)"))

    # ---- stats
    S_sb = cx.rows.tile([128, C], f32, tag="S")
    nc.vector.tensor_copy(S_sb, cx.psSs[s])
    pst = cx.psSC.tile([128, C], f32, tag="st")
    nc.tensor.transpose(pst, S_sb, cx.ident_f)
    Scol = cx.chain.tile([128, 1], f32, tag="Scol")
    nc.vector.tensor_copy(Scol, pst[:, 0:1])

    scr = cx.rows.tile([128, C], f32, tag="scr")
    ss = cx.chain.tile([128, 1], f32, tag="ss")
    nc.vector.scalar_tensor_tensor(
        out=scr, in0=S_sb, scalar=1.0, in1=S_sb,
        op0=ALU.bypass, op1=ALU.mult, accum_out=ss,
    )
    s0sq = cx.chain.tile([128, 1], f32, tag="s0sq")
    nc.vector.tensor_scalar(
        out=s0sq, in0=S_sb[:, 0:1], scalar1=S_sb[:, 0:1], scalar2=None,
        op0=ALU.mult,
    )
    nls = cx.chain.tile([128, 1], f32, tag="nls")  # -linner(S,S) = 2*S0^2 - ss
    nc.vector.tensor_scalar(
        out=nls, in0=s0sq, scalar1=2.0, scalar2=ss, op0=ALU.mult,
        op1=ALU.subtract,
    )
    h1 = cx.chain.tile([128, 1], f32, tag="h1")  # sqrt(-linner(S,S))
    nc.scalar.activation(h1, nls, ACTF.Sqrt)
    rn = cx.chain.tile([128, 1], f32, tag="rn")  # 1/sqrt(...)
    nc.vector.reciprocal(rn, h1)
    Mrow = cx.stats.tile([128, C], f32, tag="Mrow")
    nc.vector.tensor_scalar_mul(Mrow, S_sb, rn)
    mean0 = Mrow[:, 0:1]
    t1p = cx.chain.tile([128, 1], f32, tag="t1p")
    nc.vector.tensor_scalar_add(t1p, mean0, 1.0)
    i1p = cx.stats.tile([128, 1], f32, tag="i1p")  # 1/(1+mean0)
    nc.vector.reciprocal(i1p, t1p)
    # W as a column: w = -S*rn except w[0] = +S0*rn (sign column handles it)
    wtmp = cx.chain.tile([128, 1], f32, tag="wtmp")
    nc.vector.tensor_mul(wtmp, Scol, rn)
    wtmp2 = cx.chain.tile([128, 1], f32, tag="wtmp2")
    nc.vector.tensor_mul(wtmp2, wtmp, cx.signc)
    Wbcol = cx.stats.tile([128, 1], bf16, tag="Wbcol")
    nc.vector.tensor_copy(Wbcol, wtmp2)
    cx.bulk[s] = {"Mrow": Mrow, "i1p": i1p, "x0": x0, "Wbcol": Wbcol}


def _emit_alpha_group(nc, cx, s, grp):
    """Sample s group grp: 8 PE transposes -> PSUM, one group copy to SBUF,
    8 [128ch,128pt]x[128ch,1] matmuls -> alpha columns in pa."""
    xb = cx.xb[s]
    eng = {"d": nc.vector, "a": nc.scalar, "p": nc.gpsimd}
    if grp == 0:
        cx.bulk[s]["pa"] = cx.psPA.tile([128, NT], f32, tag="pa", name="pa")
    pa = cx.bulk[s]["pa"]
    pt = cx.psT.tile([128, 8, C], bf16, tag="pt")
    for k in range(8):
        nc.tensor.transpose(pt[:, k, :], xb[:, 8 * grp + k, :], cx.ident_b)
    xbt = cx.rows.tile([128, 8, C], bf16, tag="xbt")
    e = (WARM_XBT if s <= 1 else XBT_ENGINES)[grp]
    if e == "a":
        nc.scalar.copy(
            xbt.rearrange("p a c -> p (a c)"), pt.rearrange("p a c -> p (a c)")
        )
    else:
        eng[e].tensor_copy(
            xbt.rearrange("p a c -> p (a c)"), pt.rearrange("p a c -> p (a c)")
        )
    Wbcol = cx.bulk[s]["Wbcol"]
    for k in range(8):
        t = 8 * grp + k
        nc.tensor.matmul(
            pa[:, t : t + 1], xbt[:, k, :], Wbcol, start=True, stop=True
        )


def _emit_chain_phase(nc, cx, s, phase):
    """Sample s per-point scalar chain [128, NT], split into 3 phases so
    that each engine has at most ~2 stalled ops per phase (the 4-deep
    wait queues can then bypass into following ready work)."""
    b = cx.bulk[s]
    if phase == 1:
        al = cx.chain.tile([128, NT], f32, tag="al")
        nc.vector.tensor_scalar_max(al, b["pa"], ACLIP)
        asq = cx.chain.tile([128, NT], f32, tag="asq")
        nc.vector.tensor_mul(asq, al, al)
        r2 = cx.chain.tile([128, NT], f32, tag="r2")  # 2*sqrt(alpha^2-1)
        nc.scalar.activation(r2, asq, ACTF.Sqrt, scale=4.0, bias=cx.bm4)
        rinv05 = cx.chain.tile([128, NT], f32, tag="rinv05")  # 1/(2r)
        nc.vector.reciprocal(rinv05, r2)
        z = cx.chain.tile([128, NT], f32, tag="z")  # alpha + r = e^d
        nc.vector.scalar_tensor_tensor(
            out=z, in0=r2, scalar=0.5, in1=al, op0=ALU.mult, op1=ALU.add
        )
        b.update(al=al, rinv05=rinv05, z=z)
    elif phase == 2:
        al, z = b["al"], b["z"]
        mean0 = b["Mrow"][:, 0:1]
        # d = arccosh(alpha); the activation's accumulator gives sum(d) free
        d = cx.chain.tile([128, NT], f32, tag="d")
        dsum = cx.chain.tile([128, 1], f32, tag="dsum")
        nc.scalar.activation(d, z, ACTF.Ln, accum_out=dsum)
        negu0 = cx.chain.tile([128, NT], f32, tag="negu0")  # alpha*mean0 - x0
        nc.vector.scalar_tensor_tensor(
            out=negu0, in0=al, scalar=mean0, in1=b["x0"],
            op0=ALU.mult, op1=ALU.subtract,
        )
        negC1 = cx.stats.tile([128, NT], f32, tag="negC1")
        nc.vector.scalar_tensor_tensor(
            out=negC1, in0=negu0, scalar=b["i1p"], in1=al,
            op0=ALU.mult, op1=ALU.subtract,
        )
        # var = mean(d); g = gamma/(var+eps), all on DVE (no extra hops)
        pv = cx.psSC.tile([128, 1], f32, tag="pv")
        nc.tensor.matmul(pv, cx.ones_f, dsum, start=True, stop=True)
        ve = cx.chain.tile([128, 1], f32, tag="ve")
        nc.vector.tensor_scalar(
            out=ve, in0=pv, scalar1=1.0 / N, scalar2=EPS,
            op0=ALU.mult, op1=ALU.add,
        )
        rv = cx.chain.tile([128, 1], f32, tag="rv")
        nc.vector.reciprocal(rv, ve)
        g = cx.chain.tile([128, 1], f32, tag="g")
        nc.vector.tensor_scalar_mul(g, cx.gamma_col, rv)
        b.update(d=d, negC1=negC1, g=g)
    else:
        d, g, rinv05, negC1 = b["d"], b["g"], b["rinv05"], b["negC1"]
        nu = cx.chain.tile([128, NT], f32, tag="nu")
        nc.vector.tensor_scalar_mul(nu, d, g)
        E2 = cx.chain.tile([128, NT], f32, tag="E2")  # e^nu
        nc.scalar.activation(E2, nu, ACTF.Exp)
        Ei2 = cx.chain.tile([128, NT], f32, tag="Ei2")  # e^-nu
        nc.vector.reciprocal(Ei2, E2)
        sh2 = cx.chain.tile([128, NT], f32, tag="sh2")  # 2*sinh(nu)
        nc.vector.tensor_sub(sh2, E2, Ei2)
        ch2 = cx.stats.tile([128, NT], f32, tag="ch2")  # 2*cosh(nu)
        nc.vector.tensor_add(ch2, E2, Ei2)
        A = cx.stats.tile([128, NT], f32, tag="A")  # sinh(nu)/r
        nc.vector.tensor_mul(A, sh2, rinv05)
        negB = cx.stats.tile([128, NT], f32, tag="negB")
        nc.vector.tensor_mul(negB, A, negC1)
        cx.front[s] = (b["Mrow"], negC1, A, negB, ch2)


def _emit_y_producers(nc, cx, s, gc):
    """Sample s chunk gc: amneg/tmp producers for 8 tiles (run a chunk
    ahead of the finishes so consumer engines never stall)."""
    x_sb = cx.x_sbs[s]
    Mrow, negC1, A, negB, cosh = cx.front[s]
    for t in range(8 * gc, 8 * (gc + 1)):
        mode = Y_MODES[t]
        tmp = cx.amnegp.tile([128, C], f32, tag="amneg")
        if mode == "A":
            # tmp = Mrow*negC1_t + x_t (DVE), finished by ACT scale-copy
            nc.vector.scalar_tensor_tensor(
                out=tmp[:, 1:C], in0=Mrow[:, 1:C],
                scalar=negC1[:, t : t + 1], in1=x_sb[:, t, 1:C],
                op0=ALU.mult, op1=ALU.add,
            )
        elif mode == "B":
            # Pool-only tmp via two free-broadcast TTs, ACT finish
            nc.gpsimd.tensor_tensor(
                tmp[:, 1:C], Mrow[:, 1:C],
                negC1[:, t : t + 1].broadcast_to((128, C - 1)), ALU.mult,
            )
            nc.gpsimd.tensor_add(tmp[:, 1:C], tmp[:, 1:C], x_sb[:, t, 1:C])
        else:
            ae = nc.gpsimd if mode in "NPQ" else nc.vector
            ae.tensor_scalar_mul(tmp, Mrow, negB[:, t : t + 1])
        cx.ytmp[(s, t)] = tmp


def _emit_y_finishes(nc, cx, s, gc):
    """Sample s chunk gc: y finishes in place over x_sb (cols 1:),
    col 0 <- cosh, store."""
    x_sb = cx.x_sbs[s]
    Mrow, negC1, A, negB, cosh = cx.front[s]
    ys = cx.y_d[s * N : (s + 1) * N, :].rearrange("(p t) c -> p t c", t=NT)
    for t in range(8 * gc, 8 * (gc + 1)):
        mode = Y_MODES[t]
        tmp = cx.ytmp.pop((s, t))
        if mode in "AB":
            nc.scalar.activation(
                x_sb[:, t, 1:C], tmp[:, 1:C], ACTF.Copy,
                scale=A[:, t : t + 1],
            )
        else:
            nc.vector.scalar_tensor_tensor(
                out=x_sb[:, t, 1:C], in0=x_sb[:, t, 1:C],
                scalar=A[:, t : t + 1], in1=tmp[:, 1:C],
                op0=ALU.mult, op1=ALU.add,
            )
    # col 0 <- cosh = ch2/2, then store (coarse early, fine near the tail)
    split = STORE_SPLIT if s >= SPB - 2 else 8
    for lo in range(8 * gc, 8 * (gc + 1), split):
        hi = lo + split
        nc.gpsimd.tensor_scalar(
            out=x_sb[:, lo:hi, 0:1].rearrange("p t c -> p (t c)"),
            in0=cosh[:, lo:hi],
            scalar1=0.5, scalar2=None, op0=ALU.mult,
        )
        nc.sync.dma_start(out=ys[:, lo:hi, :], in_=x_sb[:, lo:hi, :])


def build_program():
    _install_tile_patch()
    nc = bass.Bass("TRN2", debug=False)
    x_d = nc.dram_tensor("x", [SPB * N, C], f32, kind="ExternalInput").ap()
    g_d = nc.dram_tensor("gamma", [1], f32, kind="ExternalInput").ap()
    i_d = nc.dram_tensor("ident", [128, 128], bf16, kind="ExternalInput").ap()
    y_d = nc.dram_tensor("y", [SPB * N, C], f32, kind="ExternalOutput").ap()

    with tile.TileContext(nc) as tc, ExitStack() as ctx:
        cx = _Ctx()
        cx.y_d = y_d
        singles = ctx.enter_context(tc.tile_pool(name="singles", bufs=1))
        cx.xpool = ctx.enter_context(tc.tile_pool(name="x", bufs=SPB))
        cx.xbpool = ctx.enter_context(tc.tile_pool(name="xb", bufs=4))
        cx.rows = ctx.enter_context(tc.tile_pool(name="rows", bufs=4))
        cx.amnegp = ctx.enter_context(tc.tile_pool(name="amneg", bufs=20))
        cx.chain = ctx.enter_context(tc.tile_pool(name="chain", bufs=4))
        cx.stats = ctx.enter_context(tc.tile_pool(name="stats", bufs=4))
        cx.psS = ctx.enter_context(tc.tile_pool(name="psS", bufs=2, space="PSUM"))
        cx.psT = ctx.enter_context(tc.tile_pool(name="psT", bufs=2, space="PSUM"))
        cx.psPA = ctx.enter_context(tc.tile_pool(name="psPA", bufs=2, space="PSUM"))
        cx.psSC = ctx.enter_context(tc.tile_pool(name="psSC", bufs=1, space="PSUM"))

        # first x chunk goes first so the DMA pipe fills immediately; the
        # tiny constant loads then ride behind it
        xs0 = x_d[0:N, :].rearrange("(p t) c -> p t c", t=NT)
        x_sb0 = cx.xpool.tile([128, NT, C], f32, tag="xsb", name="xsb0")
        step0 = NT // S0_SPLIT
        nc.sync.dma_start(out=x_sb0[:, 0:step0, :], in_=xs0[:, 0:step0, :])
        cx.ones = singles.tile([128, 128], bf16)
        nc.vector.memset(cx.ones, 1.0)
        cx.ones_f = singles.tile([128, 128], f32)
        nc.vector.memset(cx.ones_f, 1.0)
        cx.bln05 = singles.tile([128, 1], f32)
        nc.vector.memset(cx.bln05, LN_HALF)
        cx.bm1 = singles.tile([128, 1], f32)
        nc.vector.memset(cx.bm1, -1.0)
        cx.bm4 = singles.tile([128, 1], f32)
        nc.vector.memset(cx.bm4, -4.0)

        cx.signc = singles.tile([128, 1], f32)
        nc.vector.memset(cx.signc, -1.0)
        nc.vector.memset(cx.signc[0:1, 0:1], 1.0)

        # all sample loads up-front (static buffers -> no deps, DMA streams)
        cx.x_sbs = [x_sb0]
        for i in range(1, S0_SPLIT):
            nc.sync.dma_start(
                out=x_sb0[:, i * step0 : (i + 1) * step0, :],
                in_=xs0[:, i * step0 : (i + 1) * step0, :],
            )
        # constants ride behind sample 0's loads (needed only at ~7us)
        cx.ident_b = singles.tile([128, 128], bf16)
        nc.sync.dma_start(out=cx.ident_b, in_=i_d)
        cx.gamma_col = singles.tile([128, 1], f32)
        nc.sync.dma_start(out=cx.gamma_col, in_=g_d.to_broadcast((128, 1)))
        cx.ident_f = singles.tile([128, 128], f32)
        nc.vector.tensor_copy(cx.ident_f, cx.ident_b)
        for s in range(1, SPB):
            xs = x_d[s * N : (s + 1) * N, :].rearrange("(p t) c -> p t c", t=NT)
            x_sb = cx.xpool.tile([128, NT, C], f32, tag="xsb")
            split = S1_SPLIT if s == 1 else 1
            step = NT // split
            for i in range(split):
                nc.sync.dma_start(
                    out=x_sb[:, i * step : (i + 1) * step, :],
                    in_=xs[:, i * step : (i + 1) * step, :],
                )
            cx.x_sbs.append(x_sb)

        # Fine-grained interleaved emission. Period p overlaps three samples:
        # y chunks of p-1 (producers one chunk ahead of finishes), chain
        # phases of p, conv chunks + bulk of p+1.
        cx.bulk = {}
        cx.front = {}
        cx.xb = {}
        cx.psSs = {}
        cx.ytmp = {}

        def bulk_tail(s):
            _emit_stats(nc, cx, s)
            for grp in range(4):
                _emit_alpha_group(nc, cx, s, grp)

        # warmup: sample 0 front entirely
        for i in range(4):
            _emit_conv_chunk(nc, cx, 0, i)
        bulk_tail(0)
        for p in range(SPB + 1):
            # period p: y(p-1), chain(p), conv+bulk(p+1)
            have_y = 1 <= p <= SPB
            have_ch = p < SPB
            have_next = p + 1 < SPB
            if have_next:
                _emit_conv_chunk(nc, cx, p + 1, 0)
            if have_y:
                _emit_y_producers(nc, cx, p - 1, 0)
            if have_ch:
                _emit_chain_phase(nc, cx, p, 1)
            if have_next:
                _emit_conv_chunk(nc, cx, p + 1, 1)
            if have_y:
                _emit_y_producers(nc, cx, p - 1, 1)
                _emit_y_finishes(nc, cx, p - 1, 0)
            if have_ch:
                _emit_chain_phase(nc, cx, p, 2)
            if have_next:
                _emit_conv_chunk(nc, cx, p + 1, 2)
            if have_y:
                _emit_y_producers(nc, cx, p - 1, 2)
                _emit_y_finishes(nc, cx, p - 1, 1)
            if have_ch:
                _emit_chain_phase(nc, cx, p, 3)
            if have_next:
                _emit_conv_chunk(nc, cx, p + 1, 3)
            if have_y:
                _emit_y_producers(nc, cx, p - 1, 3)
                _emit_y_finishes(nc, cx, p - 1, 2)
                _emit_y_finishes(nc, cx, p - 1, 3)
            if have_next:
                bulk_tail(p + 1)
    return nc


_PROGRAM = None


def _get_program():
    global _PROGRAM
    if _PROGRAM is None:
        _PROGRAM = build_program()
    return _PROGRAM


def _numpy_reference(x, beta, gamma):
    """Full-precision numpy fallback (general beta)."""
    CLAMP = 1e-8
    bs, h, w, c = x.shape
    x = x.reshape(bs, h * w, c).astype(np.float64)
    beta = beta.astype(np.float64)
    e0 = np.zeros(c)
    e0[0] = 1.0

    def linner(a, b):
        return (a * b).sum(-1, keepdims=True) - 2.0 * a[..., :1] * b[..., :1]

    m = x.mean(1, keepdims=True)
    mean = m / np.sqrt(np.clip(-linner(m, m), CLAMP, None))
    alpha = np.clip(-linner(mean, x), 1.0 + 1e-7, None)
    u = x - alpha * mean
    un = np.sqrt(np.clip(linner(u, u), CLAMP, None))
    x_T = np.arccosh(alpha) * u / un
    x_T = x_T - (x_T[..., :1] / (1.0 + mean[..., :1])) * (mean + e0)
    var = np.linalg.norm(x_T, axis=-1).mean(1)[:, None, None]
    x_T = x_T * (gamma.astype(np.float64) / (var + EPS))
    x_T = x_T + (linner(beta, x_T) / (1.0 + beta[0])) * (beta + e0)
    nu = np.sqrt(np.clip(linner(x_T, x_T), CLAMP, None))
    out = np.cosh(nu) * beta + np.sinh(nu) * x_T / nu
    return out.reshape(bs, h, w, c).astype(np.float32)


def kernel(x, beta, gamma):
    x = np.ascontiguousarray(x, dtype=np.float32)
    beta = np.asarray(beta, dtype=np.float32)
    gamma = np.asarray(gamma, dtype=np.float32).reshape(1)

    e0 = np.zeros(C, np.float32)
    e0[0] = 1.0
    if not np.array_equal(beta, e0):
        return _numpy_reference(x, beta, gamma)

    from concourse.bass_utils import run_bass_kernel_spmd

    import ml_dtypes

    nc = _get_program()
    xr = x.reshape(BS * N, C)
    ident = np.eye(128, dtype=ml_dtypes.bfloat16)
    in_maps = [
        {"x": xr[s * SPB * N : (s + 1) * SPB * N], "gamma": gamma, "ident": ident}
        for s in range(NCORES)
    ]
    res = run_bass_kernel_spmd(nc, in_maps, core_ids=list(range(NCORES)))
    y = np.concatenate([r["y"] for r in res.results], axis=0)
    return y.reshape(BS, H, W, C)

